# revision 1
# baseline (speedup 1.0000x reference)
"""Complex attention (split re/im softmax) on 8 trn2 NeuronCores.

Math per (b,h) pair (L=S=1024, E=D=64):
  scores_re[l,s] = sum_e qr[l,e]kr[s,e] + qi[l,e]ki[s,e]   (x 1/sqrt(E))
  scores_im[l,s] = sum_e qi[l,e]kr[s,e] - qr[l,e]ki[s,e]   (x 1/sqrt(E))
  Ar = softmax_s(scores_re); Ai = softmax_s(scores_im)
  Vre = Ar@vr - Ai@vi ; Vim = Ar@vi + Ai@vr

Kernel strategy (per core: 4 of the 32 (b,h) pairs):
  - Pack the re/im contraction into K=128 matmuls:
      qcat = [qr; qi]^T * scale   [128, L]
      kre  = [kr; ki]^T           [128, S]
      kim  = [-ki; kr]^T          [128, S]
    scoresT (s on partitions, l free) = kre_chunk.T @ qcat, kim_chunk.T @ qcat
  - exp on ScalarE (no max-subtraction needed: |scaled scores| < ~10),
    writing P^T = exp(scoresT) as bf16.
  - AV: for each l-chunk of 128, accumulate over the 8 s-tiles:
      psum[l,0:129] += P^T_chunk.T @ [vr | vi | ones]
    The ones column makes col 128 the softmax denominator Z[l].
  - Normalize on VectorE: V = (Pr@[vr|vi])/Zr -+ (Pi@[vi|vr])/Zi combos.
"""

import numpy as np
import ml_dtypes

import concourse.bass as bass
from concourse import mybir
from concourse.tile import TileContext
from concourse.bass_utils import run_bass_kernel_spmd

B, L, H, E = 4, 1024, 8, 64
S, D = 1024, 64
NCORES = 8
PAIRS = B * H              # 32 (b,h) pairs
PPC = PAIRS // NCORES      # 4 pairs per core
NT = S // 128              # 8 s-tiles
NL = L // 128              # 8 l-chunks
VW = 132                   # padded vaug width (vr 64 | vi 64 | ones 1 | pad 3)

BF16 = mybir.dt.bfloat16
F32 = mybir.dt.float32
AF = mybir.ActivationFunctionType
ALU = mybir.AluOpType


def _split_excess_waits(nc, max_waits=1):
    """This toolchain's walrus accepts at most one sync wait per
    instruction; Tile's scheduler emits up to ~3. Move excess waits onto
    preceding same-engine nofuse NoOps (pure dispatch delay, semantics
    preserved)."""
    nsplit = 0
    for f in nc.m.functions:
        for blk in f.blocks:
            insts = list(blk.instructions)
            new = []
            changed = False
            for inst in insts:
                si = inst.sync_info
                if si is not None and si.on_wait and len(si.on_wait) > max_waits:
                    waits = list(si.on_wait)
                    excess = waits[:-max_waits]
                    for k in range(0, len(excess), max_waits):
                        nop = mybir.InstNoOp(
                            name=nc.get_next_instruction_name(), ins=[], outs=[]
                        )
                        nop.engine = inst.engine
                        nop.bass_nofuse = True
                        nop.sync_info = mybir.SyncInfo(
                            on_wait=excess[k : k + max_waits], on_update=[]
                        )
                        new.append(nop)
                        nsplit += 1
                    si.on_wait = waits[-max_waits:]
                    changed = True
                new.append(inst)
            if changed:
                blk.instructions = new
    return nsplit


def _build_program():
    nc = bass.Bass()
    qcat_d = nc.declare_dram_parameter("qcat", [PPC, 128, L], BF16, isOutput=False)
    kre_d = nc.declare_dram_parameter("kre", [PPC, 128, S], BF16, isOutput=False)
    kim_d = nc.declare_dram_parameter("kim", [PPC, 128, S], BF16, isOutput=False)
    vaug_d = nc.declare_dram_parameter("vaug", [PPC, 128, NT, VW], BF16, isOutput=False)
    out_d = nc.declare_dram_parameter("out", [PPC, NL, 128, 128], F32, isOutput=True)

    with TileContext(nc) as tc:
        with (
            tc.tile_pool(name="io", bufs=3) as io,
            tc.tile_pool(name="pp", bufs=2 * NT) as pp,
            tc.tile_pool(name="nrm", bufs=8) as nrm,
            tc.tile_pool(name="ob", bufs=8) as ob,
            tc.tile_pool(name="pss", bufs=1, space="PSUM") as pss,
            tc.tile_pool(name="psa", bufs=2, space="PSUM") as psa,
        ):

            def emit_av(state, c):
                """AV + normalize + store for l-chunk c of a finished pair."""
                p_tiles, va_t, pair = state
                pr = psa.tile([128, 129], F32, tag="pr")
                pi = psa.tile([128, 129], F32, tag="pi")
                for t in range(NT):
                    nc.tensor.matmul(
                        pr,
                        lhsT=p_tiles[t][:, c * 128 : (c + 1) * 128],
                        rhs=va_t[:, t, 0:129],
                        start=(t == 0),
                        stop=(t == NT - 1),
                    )
                for t in range(NT):
                    nc.tensor.matmul(
                        pi,
                        lhsT=p_tiles[t][:, L + c * 128 : L + (c + 1) * 128],
                        rhs=va_t[:, t, 0:129],
                        start=(t == 0),
                        stop=(t == NT - 1),
                    )
                # normalize: cols of pr = [Pr@vr | Pr@vi | Zr], pi likewise
                rr = nrm.tile([128, 1], F32, tag="rr")
                ri = nrm.tile([128, 1], F32, tag="ri")
                nc.vector.reciprocal(rr, pr[:, 128:129])
                nc.vector.reciprocal(ri, pi[:, 128:129])
                ti = nrm.tile([128, 128], F32, tag="ti")
                nc.vector.tensor_scalar_mul(ti, pi[:, 0:128], ri)
                o = ob.tile([128, 128], F32)
                # Vre = Pr@vr/Zr - Pi@vi/Zi ; Vim = Pr@vi/Zr + Pi@vr/Zi
                nc.vector.scalar_tensor_tensor(
                    o[:, 0:64], in0=pr[:, 0:64], scalar=rr, in1=ti[:, 64:128],
                    op0=ALU.mult, op1=ALU.subtract,
                )
                nc.vector.scalar_tensor_tensor(
                    o[:, 64:128], in0=pr[:, 64:128], scalar=rr, in1=ti[:, 0:64],
                    op0=ALU.mult, op1=ALU.add,
                )
                nc.sync.dma_start(out=out_d[pair, c], in_=o)

            prev = None
            for pair in range(PPC):
                q_t = io.tile([128, L], BF16, tag="q")
                kre_t = io.tile([128, S], BF16, tag="kre")
                kim_t = io.tile([128, S], BF16, tag="kim")
                va_t = io.tile([128, NT, VW], BF16, tag="va")
                nc.sync.dma_start(out=q_t, in_=qcat_d[pair])
                nc.sync.dma_start(out=kre_t, in_=kre_d[pair])
                nc.sync.dma_start(out=kim_t, in_=kim_d[pair])
                nc.sync.dma_start(out=va_t, in_=vaug_d[pair])
                p_tiles = []
                for t in range(NT):
                    ps = pss.tile([128, 2 * L], F32)  # re: 0:L, im: L:2L
                    ks = kre_t[:, t * 128 : (t + 1) * 128]
                    ki = kim_t[:, t * 128 : (t + 1) * 128]
                    for h in range(L // 512):
                        nc.tensor.matmul(
                            ps[:, h * 512 : (h + 1) * 512],
                            lhsT=ks, rhs=q_t[:, h * 512 : (h + 1) * 512],
                            start=True, stop=True,
                        )
                        nc.tensor.matmul(
                            ps[:, L + h * 512 : L + (h + 1) * 512],
                            lhsT=ki, rhs=q_t[:, h * 512 : (h + 1) * 512],
                            start=True, stop=True,
                        )
                    p_t = pp.tile([128, 2 * L], BF16)
                    nc.scalar.activation(out=p_t, in_=ps, func=AF.Exp)
                    p_tiles.append(p_t)
                    if prev is not None:
                        emit_av(prev, t)
                prev = (p_tiles, va_t, pair)
            for c in range(NL):
                emit_av(prev, c)

    _split_excess_waits(nc)
    return nc


_CACHED_NC = None


def _get_program():
    global _CACHED_NC
    if _CACHED_NC is None:
        _CACHED_NC = _build_program()
    return _CACHED_NC


def _prep_in_maps(inputs):
    return _prep(
        inputs["q_real"], inputs["q_imag"], inputs["k_real"], inputs["k_imag"],
        inputs["v_real"], inputs["v_imag"],
    )


def _prep(q_real, q_imag, k_real, k_imag, v_real, v_imag):
    bf16 = ml_dtypes.bfloat16
    scale = 1.0 / np.sqrt(E)

    # [B,L,H,E] -> [B,H,E,L]; pack re/im along E into 128 partitions
    qr_t = np.transpose(np.asarray(q_real, np.float32), (0, 2, 3, 1))
    qi_t = np.transpose(np.asarray(q_imag, np.float32), (0, 2, 3, 1))
    qcat = (np.concatenate([qr_t, qi_t], axis=2) * scale).astype(bf16)  # [B,H,128,L]

    kr_t = np.transpose(np.asarray(k_real, np.float32), (0, 2, 3, 1))
    ki_t = np.transpose(np.asarray(k_imag, np.float32), (0, 2, 3, 1))
    kre = np.concatenate([kr_t, ki_t], axis=2).astype(bf16)             # [B,H,128,S]
    kim = np.concatenate([-ki_t, kr_t], axis=2).astype(bf16)

    vr_t = np.transpose(np.asarray(v_real, np.float32), (0, 2, 1, 3))   # [B,H,S,D]
    vi_t = np.transpose(np.asarray(v_imag, np.float32), (0, 2, 1, 3))
    vaug = np.zeros((B, H, S, VW), np.float32)
    vaug[..., 0:D] = vr_t
    vaug[..., D : 2 * D] = vi_t
    vaug[..., 2 * D] = 1.0
    # [B,H,S,VW] -> [B,H,NT,128,VW] -> partition-major [B,H,128,NT,VW]
    vaug = np.transpose(vaug.reshape(B, H, NT, 128, VW), (0, 1, 3, 2, 4)).astype(bf16)

    qcat = qcat.reshape(PAIRS, 128, L)
    kre = kre.reshape(PAIRS, 128, S)
    kim = kim.reshape(PAIRS, 128, S)
    vaug = vaug.reshape(PAIRS, 128, NT, VW)

    in_maps = []
    for c in range(NCORES):
        sl = slice(c * PPC, (c + 1) * PPC)
        in_maps.append(
            {
                "qcat": np.ascontiguousarray(qcat[sl]),
                "kre": np.ascontiguousarray(kre[sl]),
                "kim": np.ascontiguousarray(kim[sl]),
                "vaug": np.ascontiguousarray(vaug[sl]),
            }
        )
    return in_maps


def kernel(q_real, q_imag, k_real, k_imag, v_real, v_imag, attn_mask=None):
    in_maps = _prep(q_real, q_imag, k_real, k_imag, v_real, v_imag)
    nc = _get_program()
    res = run_bass_kernel_spmd(nc, in_maps, list(range(NCORES)))
    outs = np.concatenate(
        [res.results[c]["out"] for c in range(NCORES)], axis=0
    )  # [32, NL, 128, 128]
    outs = outs.reshape(B, H, L, 128)
    v_re = np.transpose(outs[..., 0:D], (0, 2, 1, 3))   # [B,L,H,D]
    v_im = np.transpose(outs[..., D : 2 * D], (0, 2, 1, 3))
    return np.stack([v_re, v_im], axis=0).astype(np.float32)



# revision 7
# speedup vs baseline: 1.6428x; 1.6428x over previous
"""Complex attention (split re/im softmax) on 8 trn2 NeuronCores.

Math per (b,h) pair (L=S=1024, E=D=64):
  scores_re[l,s] = sum_e qr[l,e]kr[s,e] + qi[l,e]ki[s,e]   (x 1/sqrt(E))
  scores_im[l,s] = sum_e qi[l,e]kr[s,e] - qr[l,e]ki[s,e]   (x 1/sqrt(E))
  Ar = softmax_s(scores_re); Ai = softmax_s(scores_im)
  Vre = Ar@vr - Ai@vi ; Vim = Ar@vi + Ai@vr

Kernel strategy (per core: 4 of the 32 (b,h) pairs):
  - Pack the re/im contraction into K=128 matmuls:
      qcat = [qr; qi]^T * scale   [128, L]
      kre  = [kr; ki]^T           [128, S]
      kim  = [-ki; kr]^T          [128, S]
    scoresT (s on partitions, l free) = kre_chunk.T @ qcat, kim_chunk.T @ qcat
    written as separate re/im PSUM slices [128, 1024] (2 banks each, 3 in
    rotation) so the tensor engine never waits long on exp draining PSUM.
  - exp: split across ScalarE (true Exp activation) and DVE (Schraudolph
    fast-exp: i16 = trunc(x*2^7/ln2 + magic) bit-cast as bf16; ~1.8% rms).
    Writing P^T as bf16. No max-subtraction (|scaled scores| < ~10).
  - AV: for each l-chunk of 128, accumulate over the 8 s-tiles into a single
    PSUM bank [128, 258]:
      av[:, 0:129]   += Pr^T_chunk.T @ [vr | vi | ones]
      av[:, 129:258] += Pi^T_chunk.T @ [vr | vi | ones]
    The ones column makes cols 128/257 the softmax denominators Zr/Zi.
  - No on-chip normalization: av is DMA'd to DRAM raw; the host divides by
    Z and forms Vre/Vim during unshard (O(L*D) work, negligible).
"""

import numpy as np
import ml_dtypes

import concourse.bass as bass
from concourse import mybir
from concourse.tile import TileContext
from concourse.bass_utils import run_bass_kernel_spmd

B, L, H, E = 4, 1024, 8, 64
S, D = 1024, 64
NCORES = 8
PAIRS = B * H              # 32 (b,h) pairs
PPC = PAIRS // NCORES      # 4 pairs per core
NT = S // 128              # 8 s-tiles
NL = L // 128              # 8 l-chunks
VW = 132                   # padded vaug width (vr 64 | vi 64 | ones 1 | pad 3)

BF16 = mybir.dt.bfloat16
F32 = mybir.dt.float32
I16 = mybir.dt.int16
AF = mybir.ActivationFunctionType
ALU = mybir.AluOpType

# Schraudolph fast-exp constants for bf16 output (see module docstring).
FEXP_A = 184.6649652          # 2^7 / ln 2
FEXP_B = 16256.0 - 7.5 + 0.5  # 127*2^7 - c, +0.5 compensates trunc-to-zero

# Which exp slices the DVE takes, per pair: set of (t, part); part 0=re 1=im.
# Later pairs shift more onto DVE so the last pair's exp doesn't tail-stall
# the tensor engine's final AV block.
DVE_SLICES = [
    {(1, 1), (3, 1), (5, 1), (7, 1)},
    {(1, 1), (3, 1), (5, 1), (7, 1)},
    {(1, 1), (3, 1), (5, 1), (7, 1)},
    {(t, 1) for t in range(NT)},
]


def _split_excess_waits(nc, max_waits=1):
    """This toolchain's walrus accepts at most one sync wait per
    instruction; Tile's scheduler emits up to ~3. Move excess waits onto
    preceding same-engine nofuse NoOps (pure dispatch delay, semantics
    preserved)."""
    nsplit = 0
    for f in nc.m.functions:
        for blk in f.blocks:
            insts = list(blk.instructions)
            new = []
            changed = False
            for inst in insts:
                si = inst.sync_info
                if si is not None and si.on_wait and len(si.on_wait) > max_waits:
                    waits = list(si.on_wait)
                    excess = waits[:-max_waits]
                    for k in range(0, len(excess), max_waits):
                        nop = mybir.InstNoOp(
                            name=nc.get_next_instruction_name(), ins=[], outs=[]
                        )
                        nop.engine = inst.engine
                        nop.bass_nofuse = True
                        nop.sync_info = mybir.SyncInfo(
                            on_wait=excess[k : k + max_waits], on_update=[]
                        )
                        new.append(nop)
                        nsplit += 1
                    si.on_wait = waits[-max_waits:]
                    changed = True
                new.append(inst)
            if changed:
                blk.instructions = new
    return nsplit


def _build_program():
    nc = bass.Bass()
    qcat_d = nc.declare_dram_parameter("qcat", [PPC, 128, L], BF16, isOutput=False)
    kre_d = nc.declare_dram_parameter("kre", [PPC, 128, S], BF16, isOutput=False)
    kim_d = nc.declare_dram_parameter("kim", [PPC, 128, S], BF16, isOutput=False)
    vaug_d = nc.declare_dram_parameter("vaug", [PPC, 128, NT, VW], BF16, isOutput=False)
    # raw AV numerators + Z columns; host normalizes
    out_d = nc.declare_dram_parameter("out", [PPC, NL, 128, 258], BF16, isOutput=True)

    with TileContext(nc) as tc:
        with (
            tc.tile_pool(name="io", bufs=3) as io,
            tc.tile_pool(name="pp", bufs=2 * NT) as pp,
            tc.tile_pool(name="ps", bufs=3, space="PSUM") as ps,
            tc.tile_pool(name="psa", bufs=2, space="PSUM") as psa,
            tc.tile_pool(name="ob", bufs=4) as ob,
        ):

            def emit_av(state, c):
                """AV + store for l-chunk c of a finished pair."""
                p_tiles, va_t, pair = state
                av = psa.tile([128, 258], F32, tag="av")
                for t in range(NT):
                    nc.tensor.matmul(
                        av[:, 0:129],
                        lhsT=p_tiles[t][:, c * 128 : (c + 1) * 128],
                        rhs=va_t[:, t, 0:129],
                        start=(t == 0),
                        stop=(t == NT - 1),
                    )
                for t in range(NT):
                    nc.tensor.matmul(
                        av[:, 129:258],
                        lhsT=p_tiles[t][:, L + c * 128 : L + (c + 1) * 128],
                        rhs=va_t[:, t, 0:129],
                        start=(t == 0),
                        stop=(t == NT - 1),
                    )
                # DMA can't read PSUM (and GPSIMD can't either): bounce
                # through SBUF on DVE, downcasting to bf16 (host divides by Z
                # in f32; quantizing numerator+Z costs ~0.3% rel err).
                o = ob.tile([128, 258], BF16, tag="o")
                nc.vector.tensor_scalar(
                    out=o, in0=av, scalar1=0.0, scalar2=None, op0=ALU.add
                )
                nc.sync.dma_start(out=out_d[pair, c], in_=o)

            def emit_exp(ps_slice, p_slice, use_dve):
                if use_dve:
                    nc.vector.tensor_scalar(
                        out=p_slice.bitcast(I16),
                        in0=ps_slice,
                        scalar1=FEXP_A,
                        scalar2=FEXP_B,
                        op0=ALU.mult,
                        op1=ALU.add,
                    )
                else:
                    nc.scalar.activation(out=p_slice, in_=ps_slice, func=AF.Exp)

            prev = None
            for pair in range(PPC):
                q_t = io.tile([128, L], BF16, tag="q")
                kre_t = io.tile([128, S], BF16, tag="kre")
                kim_t = io.tile([128, S], BF16, tag="kim")
                va_t = io.tile([128, NT, VW], BF16, tag="va")
                nc.sync.dma_start(out=q_t, in_=qcat_d[pair])
                nc.sync.dma_start(out=kre_t, in_=kre_d[pair])
                nc.sync.dma_start(out=kim_t, in_=kim_d[pair])
                nc.sync.dma_start(out=va_t, in_=vaug_d[pair])
                p_tiles = []
                for t in range(NT):
                    ks = kre_t[:, t * 128 : (t + 1) * 128]
                    ki = kim_t[:, t * 128 : (t + 1) * 128]
                    p_t = pp.tile([128, 2 * L], BF16)  # re: 0:L, im: L:2L
                    ps_re = ps.tile([128, L], F32, tag="s")
                    for h in range(L // 512):
                        nc.tensor.matmul(
                            ps_re[:, h * 512 : (h + 1) * 512],
                            lhsT=ks, rhs=q_t[:, h * 512 : (h + 1) * 512],
                            start=True, stop=True,
                        )
                    emit_exp(ps_re, p_t[:, 0:L], (t, 0) in DVE_SLICES[pair])
                    ps_im = ps.tile([128, L], F32, tag="s")
                    for h in range(L // 512):
                        nc.tensor.matmul(
                            ps_im[:, h * 512 : (h + 1) * 512],
                            lhsT=ki, rhs=q_t[:, h * 512 : (h + 1) * 512],
                            start=True, stop=True,
                        )
                    emit_exp(ps_im, p_t[:, L : 2 * L], (t, 1) in DVE_SLICES[pair])
                    p_tiles.append(p_t)
                    if prev is not None:
                        emit_av(prev, t)
                prev = (p_tiles, va_t, pair)
            for c in range(NL):
                emit_av(prev, c)

    _split_excess_waits(nc)
    return nc


_CACHED_NC = None


def _get_program():
    global _CACHED_NC
    if _CACHED_NC is None:
        _CACHED_NC = _build_program()
    return _CACHED_NC


def _prep_in_maps(inputs):
    return _prep(
        inputs["q_real"], inputs["q_imag"], inputs["k_real"], inputs["k_imag"],
        inputs["v_real"], inputs["v_imag"],
    )


def _prep(q_real, q_imag, k_real, k_imag, v_real, v_imag):
    bf16 = ml_dtypes.bfloat16
    scale = 1.0 / np.sqrt(E)

    # [B,L,H,E] -> [B,H,E,L]; pack re/im along E into 128 partitions
    qr_t = np.transpose(np.asarray(q_real, np.float32), (0, 2, 3, 1))
    qi_t = np.transpose(np.asarray(q_imag, np.float32), (0, 2, 3, 1))
    qcat = (np.concatenate([qr_t, qi_t], axis=2) * scale).astype(bf16)  # [B,H,128,L]

    kr_t = np.transpose(np.asarray(k_real, np.float32), (0, 2, 3, 1))
    ki_t = np.transpose(np.asarray(k_imag, np.float32), (0, 2, 3, 1))
    kre = np.concatenate([kr_t, ki_t], axis=2).astype(bf16)             # [B,H,128,S]
    kim = np.concatenate([-ki_t, kr_t], axis=2).astype(bf16)

    vr_t = np.transpose(np.asarray(v_real, np.float32), (0, 2, 1, 3))   # [B,H,S,D]
    vi_t = np.transpose(np.asarray(v_imag, np.float32), (0, 2, 1, 3))
    vaug = np.zeros((B, H, S, VW), np.float32)
    vaug[..., 0:D] = vr_t
    vaug[..., D : 2 * D] = vi_t
    vaug[..., 2 * D] = 1.0
    # [B,H,S,VW] -> [B,H,NT,128,VW] -> partition-major [B,H,128,NT,VW]
    vaug = np.transpose(vaug.reshape(B, H, NT, 128, VW), (0, 1, 3, 2, 4)).astype(bf16)

    qcat = qcat.reshape(PAIRS, 128, L)
    kre = kre.reshape(PAIRS, 128, S)
    kim = kim.reshape(PAIRS, 128, S)
    vaug = vaug.reshape(PAIRS, 128, NT, VW)

    in_maps = []
    for c in range(NCORES):
        sl = slice(c * PPC, (c + 1) * PPC)
        in_maps.append(
            {
                "qcat": np.ascontiguousarray(qcat[sl]),
                "kre": np.ascontiguousarray(kre[sl]),
                "kim": np.ascontiguousarray(kim[sl]),
                "vaug": np.ascontiguousarray(vaug[sl]),
            }
        )
    return in_maps


def kernel(q_real, q_imag, k_real, k_imag, v_real, v_imag, attn_mask=None):
    in_maps = _prep(q_real, q_imag, k_real, k_imag, v_real, v_imag)
    nc = _get_program()
    res = run_bass_kernel_spmd(nc, in_maps, list(range(NCORES)))
    outs = np.concatenate(
        [res.results[c]["out"].astype(np.float32) for c in range(NCORES)], axis=0
    )  # [32, NL, 128, 258]
    outs = outs.reshape(B, H, L, 258)
    pr = outs[..., 0:128]
    zr = outs[..., 128:129]
    pi = outs[..., 129:257]
    zi = outs[..., 257:258]
    v_re = pr[..., 0:D] / zr - pi[..., D : 2 * D] / zi     # [B,H,L,D]
    v_im = pr[..., D : 2 * D] / zr + pi[..., 0:D] / zi
    v_re = np.transpose(v_re, (0, 2, 1, 3))                # [B,L,H,D]
    v_im = np.transpose(v_im, (0, 2, 1, 3))
    return np.stack([v_re, v_im], axis=0).astype(np.float32)
